# revision 1
# baseline (speedup 1.0000x reference)
"""Trainium2 Bass kernel for a 5-layer stacked LSTM (T=1024, B=32, H=768) + projection/log_softmax.

Strategy (v0): data-parallel over batch across the 8 NeuronCores (B_LOC=4 per
core), zero cross-core communication.  Per core, each LSTM layer runs as:
  - chunked input-projection ("xg") matmuls: xg = W_ih.T-blocks @ h_prev-chunk
    (bias folded in via an extra contraction tile carrying a constant-one row),
  - a sequential 16-step scan per chunk with the recurrent matmul in
    "weight-stationary transposed" form: gates.T[128*m-tile, B] accumulated
    over 6 K-tiles of hT(t-1), so the elementwise chain directly produces
    hT(t) with no per-step transposes.
All matmuls in bf16 (fp32 streams at 1/4 rate on the PE; bf16 weight loads get
FWL), fp32 PSUM accumulation and fp32 cell state.  Tile handles scheduling.
"""
import sys
import os

sys.path.insert(0, "/opt/trn_rl_repo")

import numpy as np
import ml_dtypes
from contextlib import ExitStack

import concourse.bass as bass
import concourse.bacc as bacc
import concourse.mybir as mybir
from concourse.tile import TileContext
from concourse.bass_utils import run_bass_kernel_spmd

BF16 = mybir.dt.bfloat16
F32 = mybir.dt.float32
Act = mybir.ActivationFunctionType
Alu = mybir.AluOpType

T_FULL = 1024
B_FULL = 32
NCORES = 8
B_LOC = B_FULL // NCORES          # 4
H = 768
G = 4 * H                         # 3072
KT = 6                            # K tiles over H
MT = 24                           # M tiles over 4H
D0 = 512                          # layer-0 input dim
V = 41                            # vocab
CHUNK = 16                        # timesteps per chunk
CC = CHUNK * B_LOC                # 64 cols per chunk


def build_program(T=T_FULL, layers=5):
    nchunk = T // CHUNK
    COLS = T * B_LOC              # hT buffer cols
    nc = bacc.Bacc(None, target_bir_lowering=False)

    xt = nc.declare_dram_parameter("xt", [128, KT * COLS], BF16, isOutput=False)
    whh_d = [nc.declare_dram_parameter(f"whh{l}", [128, KT * MT * 128], BF16, isOutput=False)
             for l in range(layers)]
    wih_d = [nc.declare_dram_parameter(f"wih{l}", [128, (KT + 1) * MT * 128], BF16, isOutput=False)
             for l in range(layers)]
    wp_d = nc.declare_dram_parameter("wp", [128, KT * V], BF16, isOutput=False)
    bp_d = nc.declare_dram_parameter("bp", [1, V], BF16, isOutput=False)
    out_d = nc.declare_dram_parameter("out", [T * B_LOC, V], F32, isOutput=True)

    es = ExitStack()
    bufA = es.enter_context(nc.sbuf_tensor("bufA", [128, KT * COLS], BF16))
    bufB = es.enter_context(nc.sbuf_tensor("bufB", [128, KT * COLS], BF16))
    whh_s = es.enter_context(nc.sbuf_tensor("whh_s", [128, KT * MT * 128], BF16))
    wih_s = es.enter_context(nc.sbuf_tensor("wih_s", [128, (KT + 1) * MT * 128], BF16))
    wp_s = es.enter_context(nc.sbuf_tensor("wp_s", [128, KT * V], BF16))
    bp_s = es.enter_context(nc.sbuf_tensor("bp_s", [1, V], BF16))
    ones_s = es.enter_context(nc.sbuf_tensor("ones_s", [1, 128], BF16))
    stg_in = es.enter_context(nc.sbuf_tensor("stg_in", [128, (KT + 1) * CC], BF16))
    stg_out = es.enter_context(nc.sbuf_tensor("stg_out", [128, KT * CC], BF16))
    xg_s = es.enter_context(nc.sbuf_tensor("xg_s", [128, MT * CC], BF16))
    sb_if = es.enter_context(nc.sbuf_tensor("sb_if", [128, 12 * B_LOC], F32))
    sb_g = es.enter_context(nc.sbuf_tensor("sb_g", [128, 6 * B_LOC], F32))
    sb_o = es.enter_context(nc.sbuf_tensor("sb_o", [128, 6 * B_LOC], F32))
    sb_ig = es.enter_context(nc.sbuf_tensor("sb_ig", [128, 6 * B_LOC], F32))
    sb_tc = es.enter_context(nc.sbuf_tensor("sb_tc", [128, 6 * B_LOC], F32))
    ct = es.enter_context(nc.sbuf_tensor("ct", [128, 6 * B_LOC], F32))
    e_s = es.enter_context(nc.sbuf_tensor("e_s", [128, V], F32))
    red_s = es.enter_context(nc.sbuf_tensor("red_s", [128, 4], F32))
    logit_s = es.enter_context(nc.sbuf_tensor("logit_s", [128, (COLS // 128) * V], F32))
    ps_xg = es.enter_context(nc.psum_tensor([128, MT * CC], F32))
    ps_g = es.enter_context(nc.psum_tensor([128, MT * B_LOC], F32))
    ps_p = es.enter_context(nc.psum_tensor([128, V], F32))

    NB = B_LOC

    with TileContext(nc) as tc:
        nc.gpsimd.partition_id()  # keep partition-id tensor alive

        nc.sync.dma_start(out=bufA[:, :], in_=xt[:, :])
        nc.sync.dma_start(out=wp_s[:, :], in_=wp_d[:, :])
        nc.sync.dma_start(out=bp_s[:, :], in_=bp_d[:, :])
        nc.gpsimd.memset(ones_s[:, :], 1.0)
        nc.gpsimd.memset(stg_in[:, :], 0.0)
        nc.gpsimd.memset(stg_in[0:1, KT * CC:(KT + 1) * CC], 1.0)

        for l in range(layers):
            ht_in, ht_out = (bufA, bufB) if l % 2 == 0 else (bufB, bufA)
            hin_v = ht_in[:, :].rearrange("p (k c) -> p k c", k=KT)
            hout_v = ht_out[:, :].rearrange("p (k c) -> p k c", k=KT)
            nc.sync.dma_start(out=whh_s[:, :], in_=whh_d[l][:, :])
            nc.sync.dma_start(out=wih_s[:, :], in_=wih_d[l][:, :])
            nc.gpsimd.memset(ct[:, :], 0.0)
            nc.gpsimd.memset(stg_out[:, :], 0.0)

            with tc.For_i(0, nchunk, 1,
                          hint_engines=(mybir.EngineType.PE, mybir.EngineType.DVE,
                                        mybir.EngineType.Activation, mybir.EngineType.SP,
                                        mybir.EngineType.Pool)) as i:
                nc.sync.dma_start(
                    out=stg_in[:, 0:KT * CC].rearrange("p (k c) -> p k c", k=KT),
                    in_=hin_v[:, :, bass.ts(i, CC)],
                )
                # xg = [W_ih.T | bias-row] blocks @ stg_in   -> ps_xg [128, MT*CC]
                for m in range(MT):
                    for k in range(KT + 1):
                        nc.tensor.matmul(
                            ps_xg[:, m * CC:(m + 1) * CC],
                            wih_s[:, (k * MT + m) * 128:(k * MT + m + 1) * 128],
                            stg_in[:, k * CC:(k + 1) * CC],
                            start=(k == 0), stop=(k == KT),
                            skip_group_check=True,
                        )
                nc.vector.tensor_copy(xg_s[:, :], ps_xg[:, :])

                xg_v = xg_s[:, :].rearrange("p (m c) -> p m c", m=MT)
                so_v = stg_out[:, :].rearrange("p (k c) -> p k c", k=KT)
                for t in range(CHUNK):
                    rcol = (t - 1) % CHUNK * NB  # t=0 reads last col of prev chunk
                    for k in range(KT):
                        for m in range(MT):
                            nc.tensor.matmul(
                                ps_g[:, m * NB:(m + 1) * NB],
                                whh_s[:, (k * MT + m) * 128:(k * MT + m + 1) * 128],
                                stg_out[:, k * CC + rcol:k * CC + rcol + NB],
                                start=(k == 0), stop=(k == KT - 1),
                                skip_group_check=True,
                            )
                    psg_v = ps_g[:, :].rearrange("p (m c) -> p m c", m=MT)
                    nc.vector.tensor_tensor(psg_v, psg_v, xg_v[:, :, t * NB:(t + 1) * NB], Alu.add)
                    # gates.T: i = m 0-5, f = 6-11, g = 12-17, o = 18-23
                    nc.scalar.activation(sb_if[:, :], ps_g[:, 0:12 * NB], Act.Sigmoid)
                    nc.scalar.activation(sb_g[:, :], ps_g[:, 12 * NB:18 * NB], Act.Tanh)
                    nc.scalar.activation(sb_o[:, :], ps_g[:, 18 * NB:24 * NB], Act.Sigmoid)
                    nc.vector.tensor_tensor(sb_ig[:, :], sb_if[:, 0:6 * NB], sb_g[:, :], Alu.mult)
                    nc.vector.tensor_tensor(ct[:, :], ct[:, :], sb_if[:, 6 * NB:12 * NB], Alu.mult)
                    nc.vector.tensor_tensor(ct[:, :], ct[:, :], sb_ig[:, :], Alu.add)
                    nc.scalar.activation(sb_tc[:, :], ct[:, :], Act.Tanh)
                    nc.vector.tensor_tensor(
                        so_v[:, :, t * NB:(t + 1) * NB],
                        sb_o[:, :].rearrange("p (k c) -> p k c", k=KT),
                        sb_tc[:, :].rearrange("p (k c) -> p k c", k=KT),
                        Alu.mult,
                    )
                nc.sync.dma_start(out=hout_v[:, :, bass.ts(i, CC)], in_=so_v[:, :, :])

        # projection + log_softmax from final hT buffer
        hfin = bufB if layers % 2 == 1 else bufA
        hfin_v = hfin[:, :].rearrange("p (k c) -> p k c", k=KT)
        for q in range(COLS // 128):
            for k in range(KT):
                nc.tensor.matmul(
                    ps_p[:, :],
                    hfin_v[:, k, q * 128:(q + 1) * 128],
                    wp_s[:, k * V:(k + 1) * V],
                    start=(k == 0), stop=False,
                    skip_group_check=True,
                )
            nc.tensor.matmul(ps_p[:, :], ones_s[0:1, :], bp_s[0:1, :],
                             start=False, stop=True, skip_group_check=True)
            nc.vector.tensor_reduce(red_s[:, 0:1], ps_p[:, :], mybir.AxisListType.X,
                                    Alu.max, negate=True)
            nc.scalar.activation(e_s[:, :], ps_p[:, :], Act.Exp, bias=red_s[:, 0:1])
            nc.vector.tensor_reduce(red_s[:, 1:2], e_s[:, :], mybir.AxisListType.X, Alu.add)
            nc.scalar.activation(red_s[:, 2:3], red_s[:, 1:2], Act.Ln)
            nc.vector.tensor_tensor(red_s[:, 3:4], red_s[:, 2:3], red_s[:, 0:1], Alu.subtract)
            nc.vector.tensor_scalar(logit_s[:, q * V:(q + 1) * V], ps_p[:, :],
                                    red_s[:, 3:4], None, Alu.subtract)

        out_v = out_d[:, :].rearrange("(q p) v -> p q v", p=128)
        nc.sync.dma_start(
            out=out_v,
            in_=logit_s[:, :].rearrange("p (q v) -> p q v", v=V),
        )

    es.close()
    nc.finalize()
    return nc


def _bf(a):
    return np.asarray(a, dtype=np.float32).astype(ml_dtypes.bfloat16)


def _pack_kxm(WT, ktiles, mtiles):
    """WT: [K, M] (already transposed weight).  Returns [128, ktiles*mtiles*128]
    with block (k, m) at cols (k*mtiles+m)*128."""
    K, M = ktiles * 128, mtiles * 128
    full = np.zeros((K, M), dtype=WT.dtype)
    full[:WT.shape[0], :WT.shape[1]] = WT
    blocks = full.reshape(ktiles, 128, mtiles, 128)
    return np.ascontiguousarray(
        blocks.transpose(1, 0, 2, 3).reshape(128, ktiles * mtiles * 128))


def prepare_inputs(x, W_ih0, W_ih, W_hh, b_ih, b_hh, Wp, bp, T=T_FULL, layers=5):
    """Returns in_maps (list per core)."""
    in_maps = []
    base = {}
    for l in range(layers):
        base[f"whh{l}"] = _pack_kxm(_bf(W_hh[l].T), KT, MT)
        wih_T = W_ih0.T if l == 0 else W_ih[l - 1].T       # [D, 3072]
        wih_full = np.zeros(((KT + 1) * 128, G), dtype=np.float32)
        wih_full[:wih_T.shape[0], :] = wih_T
        wih_full[KT * 128, :] = b_ih[l] + b_hh[l]          # bias row at row 768
        base[f"wih{l}"] = _pack_kxm(_bf(wih_full), KT + 1, MT)
    base["wp"] = _pack_kxm(_bf(Wp.T), KT, 1)[:, : KT * V].copy()
    # _pack_kxm pads M to 128; slice the V columns of each k block
    wpT = np.zeros((KT * 128, V), dtype=np.float32)
    wpT[:H, :] = Wp.T
    wp_pack = np.zeros((128, KT * V), dtype=ml_dtypes.bfloat16)
    for k in range(KT):
        wp_pack[:, k * V:(k + 1) * V] = _bf(wpT[k * 128:(k + 1) * 128, :])
    base["wp"] = wp_pack
    base["bp"] = _bf(bp).reshape(1, V)

    for c in range(NCORES):
        m = dict(base)
        xs = np.asarray(x[:T, c * B_LOC:(c + 1) * B_LOC, :], dtype=np.float32)  # [T, 4, D0]
        xT = xs.reshape(T * B_LOC, D0).T                   # [D0, T*4]
        xT_pad = np.zeros((KT * 128, T * B_LOC), dtype=np.float32)
        xT_pad[:D0, :] = xT
        blocks = xT_pad.reshape(KT, 128, T * B_LOC)
        m["xt"] = np.ascontiguousarray(
            blocks.transpose(1, 0, 2).reshape(128, KT * T * B_LOC)).astype(ml_dtypes.bfloat16)
        in_maps.append(m)
    return in_maps


def kernel(x, W_ih0, W_ih, W_hh, b_ih, b_hh, Wp, bp):
    x = np.asarray(x); W_ih0 = np.asarray(W_ih0); W_ih = np.asarray(W_ih)
    W_hh = np.asarray(W_hh); b_ih = np.asarray(b_ih); b_hh = np.asarray(b_hh)
    Wp = np.asarray(Wp); bp = np.asarray(bp)
    nc = build_program()
    in_maps = prepare_inputs(x, W_ih0, W_ih, W_hh, b_ih, b_hh, Wp, bp)
    res = run_bass_kernel_spmd(nc, in_maps, core_ids=list(range(NCORES)))
    out = np.empty((T_FULL, B_FULL, V), dtype=np.float32)
    for c in range(NCORES):
        out[:, c * B_LOC:(c + 1) * B_LOC, :] = res.results[c]["out"].reshape(T_FULL, B_LOC, V)
    return out

